# revision 1
# baseline (speedup 1.0000x reference)
"""Trainium2 Bass kernel: batched projective bilinear interpolation.

nn_BilinearInterpolation: X [16,384,384,64] f32, transformation [16,9] f32
-> out [16,224,224,64] f32.

Strategy: pure data parallel over batch (2 images per core on 8 cores).
Per core:
  - On-device coordinate pipeline (DVE): projective transform of a constant
    output grid, clamp/floor, bilinear weights, gather indices.
  - Per-pixel gather of two 512B chunks (2 adjacent pixels x 2 rows) via
    gpsimd indirect DMA at 256B index granularity.
  - Weighted blend: one broadcast tensor_tensor multiply + pair adds
    (split DVE/GPSIMD), store via HWDGE DMA.
"""
import numpy as np
from contextlib import ExitStack

import concourse.bass as bass
import concourse.bacc as bacc
import concourse.mybir as mybir
import concourse.tile as tile
from concourse.bass_utils import run_bass_kernel_spmd

F32 = mybir.dt.float32
I32 = mybir.dt.int32
OP = mybir.AluOpType

B, HIN, WIN, C = 16, 384, 384, 64
OUT_H = OUT_W = 224
NCORES = 8
BL = B // NCORES            # images per core
N = OUT_H * OUT_W           # 50176 output pixels per image
P = 128
COLS = N // P               # 392 pixels per partition per image
K = 28                      # pixels per partition per gather tile
T = COLS // K               # 14 gather tiles per image
IMG_ELEMS = HIN * WIN * C   # elements per image

_cache = {}


def _build_program():
    nc = bacc.Bacc("TRN2", target_bir_lowering=False, debug=False)

    Xd = nc.dram_tensor("X", [BL * HIN * WIN, C], F32, kind="ExternalInput")
    gxd = nc.dram_tensor("gx", [P, COLS], F32, kind="ExternalInput")
    gyd = nc.dram_tensor("gy", [P, COLS], F32, kind="ExternalInput")
    trd = nc.dram_tensor("trep", [BL, P, 9], F32, kind="ExternalInput")
    outd = nc.dram_tensor("out", [BL, T, P, K * C], F32, kind="ExternalOutput")

    with tile.TileContext(nc) as tc, ExitStack() as ctx:
        const_p = ctx.enter_context(tc.tile_pool(name="const", bufs=1))
        coord_p = ctx.enter_context(tc.tile_pool(name="coord", bufs=1))
        wi_p = ctx.enter_context(tc.tile_pool(name="wi", bufs=2))
        g_p = ctx.enter_context(tc.tile_pool(name="g", bufs=2))
        r_p = ctx.enter_context(tc.tile_pool(name="r", bufs=3))
        r2_p = ctx.enter_context(tc.tile_pool(name="r2", bufs=2))

        gx_t = const_p.tile([P, COLS], F32)
        nc.sync.dma_start(out=gx_t[:], in_=gxd[:])
        gy_t = const_p.tile([P, COLS], F32)
        nc.sync.dma_start(out=gy_t[:], in_=gyd[:])

        def ctile(tag):
            return coord_p.tile([P, COLS], F32, tag=tag, name=tag)

        for b in range(BL):
            tr = coord_p.tile([P, 9], F32, tag="tr")
            nc.sync.dma_start(out=tr[:], in_=trd[b])
            t00, t01, t02 = tr[:, 0:1], tr[:, 1:2], tr[:, 2:3]
            t10, t11, t12 = tr[:, 3:4], tr[:, 4:5], tr[:, 5:6]
            t20, t21 = tr[:, 6:7], tr[:, 7:8]
            t22p = coord_p.tile([P, 1], F32, tag="t22p")
            nc.vector.tensor_scalar(out=t22p[:], in0=tr[:, 8:9], scalar1=1e-6,
                                    scalar2=None, op0=OP.add)

            # homogeneous coords: xh = gx*t00 + gy*t01 + t02 (etc.)
            xh, yh, zh = ctile('xh'), ctile('yh'), ctile('zh')
            nc.vector.tensor_scalar(out=xh[:], in0=gx_t[:], scalar1=t00,
                                    scalar2=t02, op0=OP.mult, op1=OP.add)
            nc.vector.scalar_tensor_tensor(out=xh[:], in0=gy_t[:], scalar=t01,
                                           in1=xh[:], op0=OP.mult, op1=OP.add)
            nc.vector.tensor_scalar(out=yh[:], in0=gx_t[:], scalar1=t10,
                                    scalar2=t12, op0=OP.mult, op1=OP.add)
            nc.vector.scalar_tensor_tensor(out=yh[:], in0=gy_t[:], scalar=t11,
                                           in1=yh[:], op0=OP.mult, op1=OP.add)
            nc.vector.tensor_scalar(out=zh[:], in0=gx_t[:], scalar1=t20,
                                    scalar2=t22p[:], op0=OP.mult, op1=OP.add)
            nc.vector.scalar_tensor_tensor(out=zh[:], in0=gy_t[:], scalar=t21,
                                           in1=zh[:], op0=OP.mult, op1=OP.add)

            rz = ctile('rz')
            nc.vector.reciprocal(out=rz[:], in_=zh[:])

            # pixel coords: x = 192*(xh*rz) + 192; u = x - 191 (mask helper)
            u, x = ctile('u'), ctile('x')
            nc.vector.tensor_tensor(out=u[:], in0=xh[:], in1=rz[:], op=OP.mult)
            nc.vector.tensor_scalar(out=u[:], in0=u[:], scalar1=192.0,
                                    scalar2=1.0, op0=OP.mult, op1=OP.add)
            nc.vector.tensor_scalar(out=x[:], in0=u[:], scalar1=191.0,
                                    scalar2=None, op0=OP.add)
            w_, y = ctile('w_'), ctile('y')
            nc.vector.tensor_tensor(out=w_[:], in0=yh[:], in1=rz[:], op=OP.mult)
            nc.vector.tensor_scalar(out=w_[:], in0=w_[:], scalar1=192.0,
                                    scalar2=1.0, op0=OP.mult, op1=OP.add)
            nc.vector.tensor_scalar(out=y[:], in0=w_[:], scalar1=191.0,
                                    scalar2=None, op0=OP.add)

            # clamp then floor (robust to trunc or RNE float->int casts)
            sx, sy = ctile('sx'), ctile('sy')
            nc.vector.tensor_scalar(out=sx[:], in0=x[:], scalar1=0.0,
                                    scalar2=383.0, op0=OP.max, op1=OP.min)
            nc.vector.tensor_scalar(out=sy[:], in0=y[:], scalar1=0.0,
                                    scalar2=383.0, op0=OP.max, op1=OP.min)
            fxi = coord_p.tile([P, COLS], I32, tag="fxi")
            fyi = coord_p.tile([P, COLS], I32, tag="fyi")
            fxf, fyf, corr = ctile('fxf'), ctile('fyf'), ctile('corr')
            nc.vector.tensor_copy(out=fxi[:], in_=sx[:])
            nc.vector.tensor_copy(out=fxf[:], in_=fxi[:])
            nc.vector.tensor_tensor(out=corr[:], in0=fxf[:], in1=sx[:], op=OP.is_gt)
            nc.vector.tensor_tensor(out=fxf[:], in0=fxf[:], in1=corr[:], op=OP.subtract)
            nc.vector.tensor_copy(out=fyi[:], in_=sy[:])
            nc.vector.tensor_copy(out=fyf[:], in_=fyi[:])
            nc.vector.tensor_tensor(out=corr[:], in0=fyf[:], in1=sy[:], op=OP.is_gt)
            nc.vector.tensor_tensor(out=fyf[:], in0=fyf[:], in1=corr[:], op=OP.subtract)

            # neighbors and gather start column
            x1c, xs, y1c = ctile('x1c'), ctile('xs'), ctile('y1c')
            nc.vector.tensor_scalar(out=x1c[:], in0=fxf[:], scalar1=1.0,
                                    scalar2=383.0, op0=OP.add, op1=OP.min)
            nc.vector.tensor_scalar(out=xs[:], in0=fxf[:], scalar1=382.0,
                                    scalar2=None, op0=OP.min)
            nc.vector.tensor_scalar(out=y1c[:], in0=fyf[:], scalar1=1.0,
                                    scalar2=383.0, op0=OP.add, op1=OP.min)

            # lerp factors and the degenerate-clamp mask
            aq, bq, cq, dq = ctile('aq'), ctile('bq'), ctile('cq'), ctile('dq')
            nc.vector.tensor_tensor(out=aq[:], in0=x1c[:], in1=x[:], op=OP.subtract)
            nc.vector.tensor_tensor(out=bq[:], in0=x[:], in1=fxf[:], op=OP.subtract)
            nc.vector.tensor_tensor(out=cq[:], in0=y1c[:], in1=y[:], op=OP.subtract)
            nc.vector.tensor_tensor(out=dq[:], in0=y[:], in1=fyf[:], op=OP.subtract)
            # in-range mask: |u| < 192 and |w_| < 192  (via squares; 192^2
            # is exact in fp32 so the boundary cases stay exact)
            mx, mm = ctile('mx'), ctile('mm')
            nc.vector.tensor_tensor(out=mx[:], in0=u[:], in1=u[:], op=OP.mult)
            nc.vector.tensor_tensor(out=mm[:], in0=w_[:], in1=w_[:], op=OP.mult)
            nc.vector.tensor_tensor(out=mm[:], in0=mm[:], in1=mx[:], op=OP.max)
            nc.vector.tensor_scalar(out=mm[:], in0=mm[:], scalar1=float(192 * 192),
                                    scalar2=None, op0=OP.is_lt)
            wl, wr = ctile('wl'), ctile('wr')
            nc.vector.tensor_tensor(out=wl[:], in0=aq[:], in1=mm[:], op=OP.mult)
            nc.vector.tensor_tensor(out=wr[:], in0=bq[:], in1=mm[:], op=OP.mult)

            # weights in chunk order [A0, A1, B0, B1] per pixel
            W_img = wi_p.tile([P, 4 * COLS], F32, tag="W")
            Wv = W_img[:].rearrange("p (n j) -> p n j", n=COLS, j=4)
            nc.vector.tensor_tensor(out=Wv[:, :, 0], in0=wl[:], in1=cq[:], op=OP.mult)
            nc.vector.tensor_tensor(out=Wv[:, :, 1], in0=wr[:], in1=cq[:], op=OP.mult)
            nc.vector.tensor_tensor(out=Wv[:, :, 2], in0=wl[:], in1=dq[:], op=OP.mult)
            nc.vector.tensor_tensor(out=Wv[:, :, 3], in0=wr[:], in1=dq[:], op=OP.mult)

            # chunk indices (256B units): iA = y0*384 + xs, iB = y1*384 + xs
            iA, iB = ctile('iA'), ctile('iB')
            nc.vector.scalar_tensor_tensor(out=iA[:], in0=fyf[:], scalar=float(WIN),
                                           in1=xs[:], op0=OP.mult, op1=OP.add)
            nc.vector.scalar_tensor_tensor(out=iB[:], in0=y1c[:], scalar=float(WIN),
                                           in1=xs[:], op0=OP.mult, op1=OP.add)
            idx_img = wi_p.tile([P, 2 * COLS], I32, tag="idx")
            iv = idx_img[:].rearrange("p (n j) -> p n j", n=COLS, j=2)
            nc.vector.tensor_copy(out=iv[:, :, 0], in_=iA[:])
            nc.vector.tensor_copy(out=iv[:, :, 1], in_=iB[:])

            for t in range(T):
                g_t = g_p.tile([P, 2 * K * 128], F32, tag="g")
                # HW indirect DMA consumes ONE index per dest partition, so
                # issue one instruction per chunk column (128 x 512B each).
                for j in range(2 * K):
                    nc.gpsimd.indirect_dma_start(
                        out=g_t[:, j * 128:(j + 1) * 128],
                        out_offset=None,
                        in_=Xd[:],
                        in_offset=bass.IndirectOffsetOnAxis(
                            ap=idx_img[:, t * 2 * K + j:t * 2 * K + j + 1], axis=0),
                        element_offset=b * IMG_ELEMS,
                    )
                gv = g_t[:].rearrange("p (k j c) -> p k j c", k=K, j=4, c=C)
                wv = (W_img[:, t * 4 * K:(t + 1) * 4 * K]
                      .rearrange("p (k j) -> p k j", k=K, j=4)
                      .unsqueeze(3).to_broadcast([P, K, 4, C]))
                nc.vector.tensor_tensor(out=gv, in0=gv, in1=wv, op=OP.mult)

                r_t = r_p.tile([P, K * C], F32, tag="r")
                r2_t = r2_p.tile([P, K * C], F32, tag="r2")
                rv = r_t[:].rearrange("p (k c) -> p k c", k=K, c=C)
                r2v = r2_t[:].rearrange("p (k c) -> p k c", k=K, c=C)
                nc.vector.tensor_tensor(out=rv, in0=gv[:, :, 0, :],
                                        in1=gv[:, :, 1, :], op=OP.add)
                # Pool (gpsimd) is saturated by SWDGE descriptor generation
                # for the gathers, so all blend math stays on DVE.
                nc.vector.tensor_tensor(out=r2v, in0=gv[:, :, 2, :],
                                        in1=gv[:, :, 3, :], op=OP.add)
                nc.vector.tensor_tensor(out=r_t[:], in0=r_t[:], in1=r2_t[:],
                                        op=OP.add)
                nc.sync.dma_start(out=outd[b, t], in_=r_t[:])

    nc.compile()
    return nc


def _grid_constants():
    # must mirror reference: linspace over [-1,1], meshgrid, raveled
    xs = np.linspace(-1.0, 1.0, OUT_W).astype(np.float32)
    ys = np.linspace(-1.0, 1.0, OUT_H).astype(np.float32)
    xc, yc = np.meshgrid(xs, ys)
    # pixel n = t*(P*K) + p*K + k  <->  grid column c = t*K + k on partition p
    def to_tiles(a):
        return (a.ravel().reshape(T, P, K).transpose(1, 0, 2)
                .reshape(P, COLS).astype(np.float32).copy())
    return to_tiles(xc), to_tiles(yc)


def kernel(X, transformation, _trace=False):
    X = np.ascontiguousarray(X, dtype=np.float32)
    transformation = np.ascontiguousarray(transformation, dtype=np.float32)

    if "nc" not in _cache:
        _cache["nc"] = _build_program()
        _cache["grid"] = _grid_constants()
    nc = _cache["nc"]
    gx, gy = _cache["grid"]

    in_maps = []
    for i in range(NCORES):
        xb = X[i * BL:(i + 1) * BL].reshape(BL * HIN * WIN, C)
        tr = transformation[i * BL:(i + 1) * BL]  # [BL, 9]
        trep = np.broadcast_to(tr[:, None, :], (BL, P, 9)).copy()
        in_maps.append({"X": xb, "gx": gx, "gy": gy, "trep": trep})

    res = run_bass_kernel_spmd(nc, in_maps, list(range(NCORES)), trace=_trace)
    _cache["last_results"] = res

    outs = [res.results[i]["out"].reshape(BL, OUT_H, OUT_W, C)
            for i in range(NCORES)]
    return np.concatenate(outs, axis=0)

